# revision 26
# baseline (speedup 1.0000x reference)
"""Trainium2 Bass kernel for nn_Attention_24043226923261.

Per-pixel cross-attention: RMSNorm(c) -> kv proj -> softmax over N=8 context
slices with a query shared across the 32x32 spatial grid -> out proj.

Sharding: data-parallel over B=8 across the 8 NeuronCores (core b owns batch
b). Zero collectives.

Host-side weight folding (exact math, as the 100us baseline):
  - query path qh = silu(emb[q]@w1+b1)@w2+b2 is tiny ([8,512]); dots =
    c_norm @ (w_k @ qh^T), so qh, attn_scale and rms_w fold into a per-core
    [256,8] matrix wq.  k is never materialized.
  - rms_w folds into wv/wq; the per-token rsqrt(mean(c^2)) scale s[t,n] is
    applied on device (k-side in the logits, v-side in the softmax weights).
  - out proj computed transposed (out^T = wo^T @ h^T) -> channel-major
    [256, H*W] output layout directly.

v4 (from traces of the baseline and two failed restructures):
  - The combine h = sum_n a_n*v_n was the bottleneck: 64 DVE multiplies at
    ~660ns reading PSUM f32.  Now v is computed in [128,1024] two-bank PSUM
    pairs, ACT copies each pair to SBUF bf16 (~1030ns) during the otherwise
    idle load window, and the weighted multiply runs in DVE 2x_1p mode
    (~640ns per PAIR, i.e. half).  hs uses (d,e) order so the broadcast
    weight view keeps a packed stride-1 last dim (2x requirement).
  - ACT makes room by moving all squares to DVE/GPSIMD; exp is emitted
    before the last copy batch so softmax is not stuck behind it.
  - av layout is (tt, n, e) so pair-slices of the weights are legal views.
  - pair adds: first add per tile on GPSIMD, rest on DVE; folds alternate.
  - bf16 output (rel-err budget allows), halving out-DMA.
  - measured-worse ideas (do not revisit blindly): whole-tt GPSIMD add
    chains (SBUF contention halves concurrent DVE ops), per-n softmax
    pipelining with cross-engine ladders (sqrt->recip->Dsc->exp per n
    stalls every queue), direct-from-psum pair muls mid-stream (block psum
    ring slots until av exists, stalling the PE v-pipe).
"""

import sys

for _p in ("/opt/trn_rl_repo",):
    if _p not in sys.path:
        sys.path.insert(0, _p)

import numpy as np


B = 8
N = 8          # context slices (softmax axis)
NP = N // 2    # context pairs
CH = 256       # channels / hidden
H = W = 32
T = H * W      # 1024 spatial tokens per batch
HEADS = 8
HD = 64        # head dim
HS = HEADS * HD  # 512
EPS = 1e-6
NCORES = 8
PT = 128       # partition tile
TT = T // PT   # 8 token tiles
KCH = CH // PT  # 2 contraction chunks over channels
KHS = HS // PT  # 4 contraction chunks over (d, e)
GRP = 4        # token tiles per out-proj batch

SQ_GPS = (4, 6)          # squares on GPSIMD; all others on DVE
FOLD_GPS = (0, 2, 4)     # acc fold tts on GPSIMD


def _kernel_body(nc, tc, d):
    from contextlib import ExitStack

    from concourse import mybir

    AF = mybir.ActivationFunctionType
    ALU = mybir.AluOpType
    AX = mybir.AxisListType
    f32 = mybir.dt.float32
    bf16 = mybir.dt.bfloat16

    with ExitStack() as ctx:
        const = ctx.enter_context(tc.tile_pool(name="const", bufs=1))
        cpool = ctx.enter_context(tc.tile_pool(name="c", bufs=1))
        csqp = ctx.enter_context(tc.tile_pool(name="csq", bufs=2))
        sp = ctx.enter_context(tc.tile_pool(name="s", bufs=1))
        ep = ctx.enter_context(tc.tile_pool(name="e", bufs=1))
        vsb = ctx.enter_context(tc.tile_pool(name="vsb", bufs=32))
        accp = ctx.enter_context(tc.tile_pool(name="acc", bufs=8))
        tmpp = ctx.enter_context(tc.tile_pool(name="tmp", bufs=3))
        hp = ctx.enter_context(tc.tile_pool(name="h", bufs=3))
        htp = ctx.enter_context(tc.tile_pool(name="ht", bufs=2))
        outp = ctx.enter_context(tc.tile_pool(name="o", bufs=2))
        psD = ctx.enter_context(tc.tile_pool(name="psD", bufs=1, space="PSUM"))
        psM = ctx.enter_context(tc.tile_pool(name="psM", bufs=1, space="PSUM"))
        psV = ctx.enter_context(tc.tile_pool(name="psV", bufs=2, space="PSUM"))
        psT = ctx.enter_context(tc.tile_pool(name="psT", bufs=1, space="PSUM"))
        psO = ctx.enter_context(tc.tile_pool(name="psO", bufs=1, space="PSUM"))

        eps_sb = const.tile([PT, 1], f32, tag="eps", name="eps")
        nc.vector.memset(eps_sb[:], EPS)

        # ---- DMA issues ----
        c_sb = {}

        def _load_c(eng, n):
            t = cpool.tile([PT, KCH * T], bf16, tag=f"c{n}", name=f"c{n}")
            for k in range(KCH):
                eng.dma_start(t[:, k * T:(k + 1) * T],
                              d["c"][n, k * PT:(k + 1) * PT, :])
            c_sb[n] = t

        wq_sb = []
        invc_sb = []
        for k in range(KCH):
            t = const.tile([PT, HEADS], bf16, tag=f"wq{k}", name=f"wq{k}")
            nc.sync.dma_start(t[:], d["wq"][k * PT:(k + 1) * PT, :])
            wq_sb.append(t)
        for k in range(KCH):
            t = const.tile([PT, 1], bf16, tag=f"invc{k}", name=f"invc{k}")
            nc.sync.dma_start(t[:], d["invc"][k * PT:(k + 1) * PT, :])
            invc_sb.append(t)
        _load_c(nc.sync, 0)
        wv_sb = []
        for k in range(KCH):
            t = const.tile([PT, HS], bf16, tag=f"wv{k}", name=f"wv{k}")
            nc.sync.dma_start(t[:], d["wv"][k * PT:(k + 1) * PT, :])
            wv_sb.append(t)
        _load_c(nc.gpsimd, 1)
        _load_c(nc.gpsimd, 2)
        for n in range(3, N):
            _load_c(nc.sync, n)
        wo_sb = []
        for k in range(KHS):
            t = const.tile([PT, CH], bf16, tag=f"wo{k}", name=f"wo{k}")
            nc.sync.dma_start(t[:], d["wo"][k * PT:(k + 1) * PT, :])
            wo_sb.append(t)
        bo_sb = []
        for m in range(CH // PT):
            t = const.tile([PT, 1], f32, tag=f"bo{m}", name=f"bo{m}")
            nc.sync.dma_start(t[:], d["bo"][m * PT:(m + 1) * PT, :])
            bo_sb.append(t)
        eye_sb = const.tile([PT, PT], bf16, tag="eye", name="eye")
        nc.sync.dma_start(eye_sb[:], d["eye"][:, :])

        # ---- persistent state ----
        # D_ps cols (tt, n, e) so the weight pair-slices are legal views
        D_ps = psD.tile([PT, TT * N * HEADS], f32, name="D")
        Dv = D_ps[:].rearrange("p (a n e) -> p a n e", a=TT, n=N)
        # all means live in one psum bank, cols (tt, n): no ring coupling,
        # so the single late sqrt never blocks the ACT copy stream
        mean_ps = psM.tile([PT, TT * N], f32, name="mean")
        mnv = mean_ps[:].rearrange("p (a n) -> p a n", n=N)
        sq_all = sp.tile([PT, TT * N], f32, tag="sq", name="sq_all")
        s_all = sp.tile([PT, TT * N], f32, tag="s", name="s_all")

        csq = {}
        v_ps = {}
        v_sb = {}

        def _emit_square(n):
            eng = nc.gpsimd if n in SQ_GPS else nc.vector
            t = csqp.tile([PT, KCH * T], bf16,
                          tag="csq_g" if n in SQ_GPS else "csq_v",
                          name=f"csq{n}")
            eng.tensor_mul(t[:], c_sb[n][:], c_sb[n][:])
            csq[n] = t

        def _emit_dots(n):
            for tt in range(TT):
                for k in range(KCH):
                    nc.tensor.matmul(
                        Dv[:, tt, n, :],
                        c_sb[n][:, k * T + tt * PT: k * T + (tt + 1) * PT],
                        wq_sb[k][:],
                        start=(k == 0), stop=(k == KCH - 1),
                    )

        def _emit_mean(n):
            for tt in range(TT):
                for k in range(KCH):
                    nc.tensor.matmul(
                        mnv[:, tt, n:n + 1],
                        csq[n][:, k * T + tt * PT: k * T + (tt + 1) * PT],
                        invc_sb[k][:],
                        start=(k == 0), stop=(k == KCH - 1),
                    )

        def _emit_vpair(tt, p):
            ps = psV.tile([PT, 2 * HS], f32, tag="v", name=f"v{tt}_{p}")
            for nn in range(2):
                n = 2 * p + nn
                for k in range(KCH):
                    nc.tensor.matmul(
                        ps[:, nn * HS:(nn + 1) * HS],
                        c_sb[n][:, k * T + tt * PT: k * T + (tt + 1) * PT],
                        wv_sb[k][:],
                        start=(k == 0), stop=(k == KCH - 1),
                    )
            v_ps[tt, p] = ps

        def _emit_vcopy(tt, p):
            t = vsb.tile([PT, 2 * HS], bf16, tag="vsb", name=f"vsb{tt}_{p}")
            nc.scalar.copy(t[:], v_ps[tt, p][:])
            v_sb[tt, p] = t

        # ---- per-pair unnormalized softmax: P_n = exp(D_n*s_n)*s_n ----
        # computable as soon as the pair's means exist; 1/Z applied at the
        # very end, so weighted products flow behind the DMA stream.
        Dsc = ep.tile([PT, TT * N * HEADS], bf16, tag="Dsc", name="Dsc")
        Dscv = Dsc[:].rearrange("p (a n e) -> p a n e", a=TT, n=N)
        E = ep.tile([PT, TT * N * HEADS], bf16, tag="E", name="E")
        Ev = E[:].rearrange("p (a n e) -> p a n e", a=TT, n=N)
        P = ep.tile([PT, TT * N * HEADS], bf16, tag="P", name="P")
        Pv = P[:].rearrange("p (a n e) -> p a n e", a=TT, n=N)
        sqav = sq_all[:].rearrange("p (a n) -> p a n", n=N)
        sav = s_all[:].rearrange("p (a n) -> p a n", n=N)

        def _emit_pair_softmax(p):
            n0, n1 = 2 * p, 2 * p + 1
            nc.scalar.activation(sqav[:, :, n0:n1 + 1], mnv[:, :, n0:n1 + 1],
                                 AF.Sqrt, bias=eps_sb[:])
            nc.vector.reciprocal(sav[:, :, n0:n1 + 1], sqav[:, :, n0:n1 + 1])
            s_bc = sav[:, :, n0:n1 + 1] \
                .rearrange("p a (n o) -> p a n o", n=2, o=1) \
                .broadcast_to([PT, TT, 2, HEADS])
            nc.vector.tensor_mul(Dscv[:, :, n0:n1 + 1, :],
                                 Dv[:, :, n0:n1 + 1, :], s_bc)
            nc.scalar.activation(Ev[:, :, n0:n1 + 1, :],
                                 Dscv[:, :, n0:n1 + 1, :], AF.Exp)
            nc.vector.tensor_mul(Pv[:, :, n0:n1 + 1, :],
                                 Ev[:, :, n0:n1 + 1, :], s_bc)

        # ---- combine pieces (fresh outputs only: in-place DVE ops are
        # ~4x slow). t_p = v_pair*P_pair; B = t0+t1; C = t2+t3 (GPSIMD);
        # D = B+C; hu = D.lo+D.hi; h = hu * rZ (2x broadcast mul) ----
        prods = {}
        bsum = {}
        csum = {}

        def _emit_mul(tt, p):
            w_b = Pv[:, tt, 2 * p:2 * p + 2, :] \
                .rearrange("p n (o e) -> p n o e", o=1) \
                .broadcast_to([PT, 2, HD, HEADS])
            tgt = tmpp.tile([PT, 2 * HS], bf16, tag="tmp", bufs=12,
                            name=f"tmp{tt}_{p}")
            nc.vector.tensor_mul(
                tgt[:].rearrange("p (n dd e) -> p n dd e", n=2, e=HEADS),
                v_sb[tt, p][:].rearrange("p (n dd e) -> p n dd e",
                                         n=2, e=HEADS),
                w_b,
            )
            prods[tt, p] = tgt
            if p == 1:
                b = accp.tile([PT, 2 * HS], bf16, tag="b", bufs=8,
                              name=f"b{tt}")
                eng = nc.gpsimd if tt in (2, 5) else nc.vector
                eng.tensor_add(b[:], prods[tt, 0][:], prods[tt, 1][:])
                bsum[tt] = b
            elif p == 3:
                cs = accp.tile([PT, 2 * HS], bf16, tag="c", bufs=4,
                               name=f"c{tt}")
                eng = nc.gpsimd if tt != TT - 1 else nc.vector
                eng.tensor_add(cs[:], prods[tt, 2][:], prods[tt, 3][:])
                csum[tt] = cs

        # ---- pass 0 + v + weighted-product pipeline, per context pair ----
        for n in range(N):
            _emit_square(n)
            _emit_dots(n)
            _emit_mean(n)
            if n % 2 == 1:
                p = n // 2
                _emit_pair_softmax(p)
                for tt in range(TT):
                    _emit_vpair(tt, p)
                    _emit_vcopy(tt, p)
                    _emit_mul(tt, p)

        # ---- denominator and finals ----
        Z = ep.tile([PT, TT * HEADS], f32, tag="Z", name="Z")
        nc.vector.tensor_reduce(
            Z[:], E[:].rearrange("p (a n e) -> p a e n", a=TT, n=N),
            axis=AX.X, op=ALU.add)
        rZ = ep.tile([PT, TT * HEADS], bf16, tag="rZ", name="rZ")
        with nc.allow_low_precision(reason="softmax weights are bf16 anyway"):
            nc.vector.reciprocal(rZ[:], Z[:])
        rZv = rZ[:].rearrange("p (a e) -> p a e", e=HEADS)

        ht_sb = {}

        def _emit_final(tt):
            dsum = accp.tile([PT, 2 * HS], bf16, tag="d", bufs=2,
                             name=f"d{tt}")
            nc.vector.tensor_add(dsum[:], bsum[tt][:], csum[tt][:])
            hu = hp.tile([PT, HS], bf16, tag="hu", name=f"hu{tt}")
            nc.vector.tensor_add(hu[:], dsum[:, 0:HS], dsum[:, HS:2 * HS])
            rZ_b = rZv[:, tt, :].rearrange("p (o e) -> p o e", o=1) \
                                .broadcast_to([PT, HD, HEADS])
            h = hp.tile([PT, HS], bf16, tag="h", name=f"h{tt}")
            nc.vector.tensor_mul(
                h[:].rearrange("p (dd e) -> p dd e", e=HEADS),
                hu[:].rearrange("p (dd e) -> p dd e", e=HEADS),
                rZ_b,
            )
            return h

        def _emit_tr_out(tt, h):
            g = tt // GRP
            if tt % GRP == 0:
                ht_sb[g] = htp.tile([PT, KHS * GRP * PT], bf16, tag="ht",
                                    name=f"ht{g}")
            tr = psT.tile([PT, KHS * PT], bf16, tag="tr", name=f"tr{tt}")
            for m in range(KHS):
                nc.tensor.transpose(tr[:, m * PT:(m + 1) * PT],
                                    h[:, m * PT:(m + 1) * PT], eye_sb[:])
            out_view = ht_sb[g][:].rearrange(
                "p (m q c) -> p m q c", m=KHS, q=GRP)[:, :, tt % GRP, :]
            nc.scalar.copy(out_view,
                           tr[:].rearrange("p (m c) -> p m c", m=KHS))
            if tt % GRP != GRP - 1:
                return
            for m2 in range(CH // PT):
                o_ps = psO.tile([PT, GRP * PT], f32, tag="o",
                                name=f"ops{g}_{m2}")
                for k in range(KHS):
                    nc.tensor.matmul(
                        o_ps[:],
                        wo_sb[k][:, m2 * PT:(m2 + 1) * PT],
                        ht_sb[g][:, k * GRP * PT:(k + 1) * GRP * PT],
                        start=(k == 0), stop=(k == KHS - 1),
                    )
                o_sb = outp.tile([PT, GRP * PT], bf16, tag="osb",
                                 name=f"osb{g}_{m2}")
                nc.scalar.activation(o_sb[:], o_ps[:], AF.Identity,
                                     bias=bo_sb[m2][:])
                nc.sync.dma_start(
                    d["out"][m2 * PT:(m2 + 1) * PT,
                             g * GRP * PT:(g + 1) * GRP * PT],
                    o_sb[:])

        for tt in range(TT):
            h = _emit_final(tt)
            _emit_tr_out(tt, h)


def _build_nc():
    import concourse.tile as tile
    from concourse import bacc, mybir

    f32 = mybir.dt.float32
    bf16 = mybir.dt.bfloat16
    nc = bacc.Bacc(
        "TRN2",
        target_bir_lowering=False,
        debug=False,
        enable_asserts=False,
        num_devices=NCORES,
    )
    d = {
        "c": nc.dram_tensor("c", [N, CH, T], bf16, kind="ExternalInput").ap(),
        "wv": nc.dram_tensor("wv", [CH, HS], bf16, kind="ExternalInput").ap(),
        "wq": nc.dram_tensor("wq", [CH, HEADS], bf16,
                             kind="ExternalInput").ap(),
        "wo": nc.dram_tensor("wo", [HS, CH], bf16, kind="ExternalInput").ap(),
        "bo": nc.dram_tensor("bo", [CH, 1], f32, kind="ExternalInput").ap(),
        "invc": nc.dram_tensor("invc", [CH, 1], bf16,
                               kind="ExternalInput").ap(),
        "eye": nc.dram_tensor("eye", [PT, PT], bf16, kind="ExternalInput").ap(),
        "out": nc.dram_tensor("out", [CH, T], bf16, kind="ExternalOutput").ap(),
    }
    with tile.TileContext(nc) as tc:
        _kernel_body(nc, tc, d)
    nc.compile()
    return nc


_NC_CACHE = None


def _get_nc():
    global _NC_CACHE
    if _NC_CACHE is None:
        _NC_CACHE = _build_nc()
    return _NC_CACHE


def _make_in_maps(q, c, rms_w, emb, w1, b1, w2, b2, w_kv, w_out, b_out):
    q = np.asarray(q).astype(np.int64)
    c = np.asarray(c, dtype=np.float32)
    rms_w = np.asarray(rms_w, dtype=np.float32)
    emb = np.asarray(emb, dtype=np.float32)
    w1 = np.asarray(w1, dtype=np.float32)
    b1 = np.asarray(b1, dtype=np.float32)
    w2 = np.asarray(w2, dtype=np.float32)
    b2 = np.asarray(b2, dtype=np.float32)
    w_kv = np.asarray(w_kv, dtype=np.float32)
    w_out = np.asarray(w_out, dtype=np.float32)
    b_out = np.asarray(b_out, dtype=np.float32)

    # query path (tiny: 8 vectors), exact fp32 math as the reference
    qe = emb[q]                                   # [B, CH]
    x1 = qe @ w1 + b1
    h1 = x1 * (1.0 / (1.0 + np.exp(-x1)))         # silu
    qh = (h1 @ w2 + b2).reshape(B, HEADS, HD)

    wkv3 = w_kv.reshape(CH, HEADS, 2 * HD)
    w_k = wkv3[:, :, :HD]                         # [CH, HEADS, HD]
    w_v = wkv3[:, :, HD:]
    wv = (rms_w[:, None, None] * w_v)             # [CH, HEADS, HD]
    # (d, e) column order: col d*HEADS+e
    wv_de = np.ascontiguousarray(
        wv.transpose(0, 2, 1).reshape(CH, HS), dtype=np.float32)
    scale = float(HD) ** -0.5
    # wq[b, ch, e] = rms_w[ch] * scale * sum_d w_k[ch, e, d] * qh[b, e, d]
    wq_all = np.einsum("ced,bed->bce", w_k, qh).astype(np.float32)
    wq_all = wq_all * (scale * rms_w[None, :, None])

    # out proj rows reordered to (d, e): row d*HEADS+e was row e*HD+d
    wo_de = np.ascontiguousarray(
        w_out.reshape(HEADS, HD, CH).transpose(1, 0, 2).reshape(HS, CH),
        dtype=np.float32)

    import ml_dtypes
    bf = ml_dtypes.bfloat16
    shared = {
        "wv": wv_de.astype(bf),
        "wo": wo_de.astype(bf),
        "bo": np.ascontiguousarray(b_out.reshape(CH, 1), dtype=np.float32),
        "invc": np.full((CH, 1), 1.0 / CH, dtype=np.float32).astype(bf),
        "eye": np.eye(PT, dtype=np.float32).astype(bf),
    }
    in_maps = []
    for b in range(B):
        m = dict(shared)
        m["c"] = np.ascontiguousarray(c[b].reshape(N, CH, T)).astype(bf)
        m["wq"] = np.ascontiguousarray(wq_all[b]).astype(bf)
        in_maps.append(m)
    return in_maps


def _run(in_maps, **kwargs):
    from concourse import bass_utils

    nc = _get_nc()
    return bass_utils.run_bass_kernel_spmd(
        nc, in_maps, core_ids=list(range(NCORES)), **kwargs)


def kernel(q, c, rms_w, emb, w1, b1, w2, b2, w_kv, w_out, b_out):
    in_maps = _make_in_maps(q, c, rms_w, emb, w1, b1, w2, b2, w_kv, w_out,
                            b_out)
    res = _run(in_maps)
    outs = [np.asarray(res.results[b]["out"]).astype(np.float32)
            .reshape(CH, H, W) for b in range(B)]
    return np.stack(outs, axis=0)


# revision 30
# speedup vs baseline: 1.2942x; 1.2942x over previous
"""Trainium2 Bass kernel for nn_Attention_24043226923261.

Per-pixel cross-attention: RMSNorm(c) -> kv proj -> softmax over N=8 context
slices with a query shared across the 32x32 spatial grid -> out proj.

Sharding: data-parallel over B=8 across the 8 NeuronCores (core b owns batch
b). Zero collectives.

Key algebraic restructuring (host-side weight folding, exact math):
  - query path qh = silu(emb[q]@w1+b1)@w2+b2 is a [8,512] tensor; dots =
    qh . (c_norm @ w_k) = c_norm @ (w_k @ qh^T), so fold qh, attn_scale and
    rms_w into a per-core [256,8] matrix wq.  k is never materialized and the
    kv projection halves to v-only.
  - rms_w folds into wv/wq; the per-token rsqrt(mean(c^2)) scale s_n[t] is
    applied on device: on the k side inside exp() via the activation's
    per-partition scale, on the v side by folding into the softmax weights.
  - out proj is computed transposed (out^T = w_out^T @ h^T) so the result
    lands channel-major [256, H*W], which is exactly the required output
    layout.

Scheduling (the wins over the previous 104us version):
  - softmax for ALL token tiles is hoisted ahead of the v/combine loop AND
    merged into single full-width [128,512] ops (one mul/exp/reduce/recip
    chain instead of 8 per-tile chains).  It only depends on completed
    pass-0, so this costs no pipelining, and it keeps the in-order DVE
    queue free of small blockers between the heavy [128,512]
    attention-weight multiplies (DVE is the bottleneck at ~73% busy).
  - deeper tile rings (ep/hp bufs=3) let more token tiles stay in flight;
    the all-DVE add rule applies only to the true last token tile.
  Measured: 99.9-100.2us vs 104.1-105.8 for the prior version, rel err 5e-3.
  Also measured worse and reverted: all squares on ACT (108.8us - ACT is
  pass-0's serial consumer), extra adds on GPSIMD (103.2us), GPSIMD add
  tree (137.5us), buffer depths beyond 3 (100.5us).

Negative results (measured, do not revisit blindly):
  - fp8(e4m3) DoubleRow v-matmuls: 2x PE speedup but rel err 3.8e-2 (fails
    2e-2): elementwise quantization noise does not average out in
    random-sign dots.  fp8 is only safe where sums are coherent (sumsq) or
    feed the (near-uniform) softmax.
  - a full restructure (transposed streamed pass-0 + batched XBAR DMA
    transposes + n-pair multiplies + GPSIMD add tree) measured 119-137us:
    per-DMA HWDGE issue overhead (~0.6-1.4us on the sync/scalar queues),
    GPSIMD TensorTensor at ~2.3ns/elem, and cold-PE p-states ate the
    theoretical gains; GPSIMD also cannot read PSUM at all.
"""

import sys

for _p in ("/opt/trn_rl_repo",):
    if _p not in sys.path:
        sys.path.insert(0, _p)

import numpy as np


B = 8
N = 8          # context slices (softmax axis)
CH = 256       # channels / hidden
H = W = 32
T = H * W      # 1024 spatial tokens per batch
HEADS = 8
HD = 64        # head dim
HS = HEADS * HD  # 512
EPS = 1e-6
NCORES = 8
PT = 128       # partition tile
TT = T // PT   # 8 token tiles
KCH = CH // PT  # 2 contraction chunks over channels
KHS = HS // PT  # 4 contraction chunks over (head, d)
GRP = 4        # token tiles per out-proj batch


def _kernel_body(nc, tc, d):
    from contextlib import ExitStack

    from concourse import mybir

    AF = mybir.ActivationFunctionType
    ALU = mybir.AluOpType
    AX = mybir.AxisListType
    f32 = mybir.dt.float32
    bf16 = mybir.dt.bfloat16

    with ExitStack() as ctx:
        const = ctx.enter_context(tc.tile_pool(name="const", bufs=1))
        cpool = ctx.enter_context(tc.tile_pool(name="c", bufs=1))
        c2p = ctx.enter_context(tc.tile_pool(name="c2", bufs=5))
        sp = ctx.enter_context(tc.tile_pool(name="s", bufs=1))
        ep = ctx.enter_context(tc.tile_pool(name="e", bufs=3))
        avp = ctx.enter_context(tc.tile_pool(name="av", bufs=3))
        hp = ctx.enter_context(tc.tile_pool(name="h", bufs=3))
        prodp = ctx.enter_context(tc.tile_pool(name="prod", bufs=3))
        htp = ctx.enter_context(tc.tile_pool(name="ht", bufs=2))
        outp = ctx.enter_context(tc.tile_pool(name="o", bufs=2))
        psD = ctx.enter_context(tc.tile_pool(name="psD", bufs=1, space="PSUM"))
        psV = ctx.enter_context(tc.tile_pool(name="psV", bufs=3, space="PSUM"))
        psT = ctx.enter_context(tc.tile_pool(name="psT", bufs=2, space="PSUM"))
        psO = ctx.enter_context(tc.tile_pool(name="psO", bufs=2, space="PSUM"))

        # ---- constants + c loads, ordered so c[0] lands early ----
        wq_sb = []
        invc_sb = []
        for k in range(KCH):
            t = const.tile([PT, HEADS], bf16, tag=f"wq{k}", name=f"wq{k}")
            nc.sync.dma_start(t[:], d["wq"][k * PT:(k + 1) * PT, :])
            wq_sb.append(t)
            t = const.tile([PT, 1], bf16, tag=f"invc{k}", name=f"invc{k}")
            nc.sync.dma_start(t[:], d["invc"][k * PT:(k + 1) * PT, :])
            invc_sb.append(t)
        eps_sb = const.tile([PT, 1], f32, tag="eps", name="eps")
        nc.vector.memset(eps_sb[:], EPS)

        c_sb = {}
        for k in range(KCH):
            t = cpool.tile([PT, T], bf16, tag=f"c0_{k}", name=f"c0_{k}")
            nc.sync.dma_start(t[:], d["c"][0, k * PT:(k + 1) * PT, :])
            c_sb[0, k] = t

        for n in range(1, N):
            # GPSIMD (idle through pass 0) carries two early slices on its
            # own DMA rings to parallelize the load ramp
            eng = nc.gpsimd if n in (1, 2) else nc.sync
            for k in range(KCH):
                t = cpool.tile([PT, T], bf16, tag=f"c{n}_{k}", name=f"c{n}_{k}")
                eng.dma_start(t[:], d["c"][n, k * PT:(k + 1) * PT, :])
                c_sb[n, k] = t

        # pass-1 weights load after all of c: they are not needed until
        # ~40us in, and this keeps the sync queue clear for pass-0 data
        wv_sb = []
        for k in range(KCH):
            t = const.tile([PT, HS], bf16, tag=f"wv{k}", name=f"wv{k}")
            nc.sync.dma_start(t[:], d["wv"][k * PT:(k + 1) * PT, :])
            wv_sb.append(t)
        wo_sb = []
        for k in range(KHS):
            t = const.tile([PT, CH], bf16, tag=f"wo{k}", name=f"wo{k}")
            nc.sync.dma_start(t[:], d["wo"][k * PT:(k + 1) * PT, :])
            wo_sb.append(t)
        bo_sb = []
        for m in range(CH // PT):
            t = const.tile([PT, 1], f32, tag=f"bo{m}", name=f"bo{m}")
            nc.sync.dma_start(t[:], d["bo"][m * PT:(m + 1) * PT, :])
            bo_sb.append(t)
        eye_sb = const.tile([PT, PT], bf16, tag="eye", name="eye")
        nc.sync.dma_start(eye_sb[:], d["eye"][:, :])

        D_ps = psD.tile([PT, TT * HEADS * N], f32, name="D")
        Dv = D_ps[:].rearrange("p (a e n) -> p a e n", a=TT, n=N)
        # s_all[p, (tt, n)] = rsqrt(mean_n(c^2) + eps); sq_all holds sqrt
        sq_all = sp.tile([PT, TT * N], f32, tag="sq", name="sq_all")
        sqv = sq_all[:].rearrange("p (a n) -> p a n", n=N)
        s_all = sp.tile([PT, TT * N], f32, tag="s", name="s_all")
        sv = s_all[:].rearrange("p (a n) -> p a n", n=N)

        # ---- pass 0: per context slice n: squares, mean, s, dots ----
        for n in range(N):
            for tt in range(TT):
                for k in range(KCH):
                    nc.tensor.matmul(
                        Dv[:, tt, :, n],
                        c_sb[n, k][:, tt * PT:(tt + 1) * PT],
                        wq_sb[k][:],
                        start=(k == 0), stop=(k == KCH - 1),
                    )
            c2 = [c2p.tile([PT, T], bf16, tag="c2", name=f"c2_{n}_{_k}") for _k in range(KCH)]
            for k in range(KCH):
                if n % 2 == 0:
                    nc.scalar.activation(c2[k][:], c_sb[n, k][:], AF.Square)
                else:
                    nc.vector.tensor_mul(c2[k][:], c_sb[n, k][:],
                                         c_sb[n, k][:])
            mean_ps = psO.tile([PT, TT], f32, tag="o", name=f"mean{n}")
            for tt in range(TT):
                for k in range(KCH):
                    nc.tensor.matmul(
                        mean_ps[:, tt:tt + 1],
                        c2[k][:, tt * PT:(tt + 1) * PT],
                        invc_sb[k][:],
                        start=(k == 0), stop=(k == KCH - 1),
                    )
            nc.scalar.activation(sqv[:, :, n], mean_ps[:], AF.Sqrt,
                                 bias=eps_sb[:])
        nc.vector.reciprocal(s_all[:], sq_all[:])

        # ---- pass 1a: softmax for ALL token tiles as single full-width
        # [128, 512] ops.  It is gated on complete pass-0 anyway (Dv and
        # s_all finish with the last context slice), so merging the 8
        # per-tile chains into one costs no pipelining and drops ~35
        # instructions plus their semaphores from the bottleneck DVE queue.
        s_bc8 = s_all[:].rearrange("p (a o n) -> p a o n", o=1, n=N) \
                        .broadcast_to([PT, TT, HEADS, N])
        Dsc = ep.tile([PT, TT * HEADS * N], f32, tag="Dsc", name="Dsc")
        nc.vector.tensor_mul(
            Dsc[:].rearrange("p (a e n) -> p a e n", a=TT, n=N),
            Dv, s_bc8)
        E = ep.tile([PT, TT * HEADS * N], f32, tag="E", name="E")
        nc.scalar.activation(E[:], Dsc[:], AF.Exp)
        Z = ep.tile([PT, TT * HEADS], f32, tag="Z", name="Z")
        nc.vector.tensor_reduce(
            Z[:], E[:].rearrange("p (a e n) -> p a e n", a=TT, n=N),
            axis=AX.X, op=ALU.add)
        rZ = ep.tile([PT, TT * HEADS], f32, tag="rZ", name="rZ")
        nc.vector.reciprocal(rZ[:], Z[:])
        # attnv[p, e, n] = E * (1/Z) [bcast over n] * s [bcast over e]
        rZ_bc8 = rZ[:].rearrange("p (a e o) -> p a e o", o=1, e=HEADS) \
                      .broadcast_to([PT, TT, HEADS, N])
        av_big = avp.tile([PT, TT * HEADS * N], f32, tag="av", name="av")
        avb = av_big[:].rearrange("p (a e n) -> p a e n", a=TT, n=N)
        nc.vector.tensor_mul(
            avb, E[:].rearrange("p (a e n) -> p a e n", a=TT, n=N), rZ_bc8)
        nc.gpsimd.tensor_mul(avb, avb, s_bc8)
        av_tiles = [av_big[:, tt * HEADS * N:(tt + 1) * HEADS * N]
                    for tt in range(TT)]

        # ---- pass 1b: per token tile: v matmul, h, transpose, out ----
        ht_sb = None
        for tt in range(TT):
            if tt % GRP == 0:
                ht_sb = [htp.tile([PT, GRP * PT], bf16, tag=f"ht{k}", name=f"ht{k}_{tt}")
                         for k in range(KHS)]
            avv = av_tiles[tt].rearrange("p (e n) -> p e n", n=N)

            h = hp.tile([PT, HS], bf16, tag="h", name=f"h{tt}")
            for n in range(N):
                v_ps = psV.tile([PT, HS], f32, tag="v", name=f"v{tt}_{n}")
                for k in range(KCH):
                    nc.tensor.matmul(
                        v_ps[:],
                        c_sb[n, k][:, tt * PT:(tt + 1) * PT],
                        wv_sb[k][:],
                        start=(k == 0), stop=(k == KCH - 1),
                    )
                av_b = avv[:, :, n:n + 1].broadcast_to([PT, HEADS, HD])
                tgt = h if n == 0 else prodp.tile([PT, HS], bf16, tag="prod", name=f"prod{tt}_{n}")
                nc.vector.tensor_mul(
                    tgt[:].rearrange("p (e d) -> p e d", d=HD),
                    v_ps[:].rearrange("p (e d) -> p e d", d=HD),
                    av_b,
                )
                if n > 0:
                    # very last tile: keep the serial chain on the fast
                    # engine — nothing follows to hide the slow Q7 adds.
                    # (tt=3 feeds GRP-0 out-proj but tts 4-7 still overlap.)
                    if tt == TT - 1:
                        eng = nc.vector
                    else:
                        eng = nc.gpsimd if n % 2 == 0 else nc.vector
                    eng.tensor_add(h[:], h[:], tgt[:])

            for m in range(KHS):
                tr = psT.tile([PT, PT], bf16, tag="tr", name=f"tr{tt}_{m}")
                nc.tensor.transpose(tr[:], h[:, m * PT:(m + 1) * PT], eye_sb[:])
                nc.scalar.copy(
                    ht_sb[m][:, (tt % GRP) * PT:(tt % GRP + 1) * PT], tr[:])

            if tt % GRP == GRP - 1:
                g = tt // GRP
                for m2 in range(CH // PT):
                    o_ps = psO.tile([PT, GRP * PT], f32, tag="o", name=f"ops{tt}_{m2}")
                    for k in range(KHS):
                        nc.tensor.matmul(
                            o_ps[:],
                            wo_sb[k][:, m2 * PT:(m2 + 1) * PT],
                            ht_sb[k][:],
                            start=(k == 0), stop=(k == KHS - 1),
                        )
                    o_sb = outp.tile([PT, GRP * PT], bf16, tag="o", name=f"osb{tt}_{m2}")
                    nc.scalar.activation(o_sb[:], o_ps[:], AF.Identity,
                                         bias=bo_sb[m2][:])
                    nc.sync.dma_start(
                        d["out"][m2 * PT:(m2 + 1) * PT,
                                 g * GRP * PT:(g + 1) * GRP * PT],
                        o_sb[:])


def _build_nc():
    import concourse.tile as tile
    from concourse import bacc, mybir

    f32 = mybir.dt.float32
    bf16 = mybir.dt.bfloat16
    nc = bacc.Bacc(
        "TRN2",
        target_bir_lowering=False,
        debug=False,
        enable_asserts=False,
        num_devices=NCORES,
    )
    d = {
        "c": nc.dram_tensor("c", [N, CH, T], bf16, kind="ExternalInput").ap(),
        "wv": nc.dram_tensor("wv", [CH, HS], bf16, kind="ExternalInput").ap(),
        "wq": nc.dram_tensor("wq", [CH, HEADS], bf16, kind="ExternalInput").ap(),
        "wo": nc.dram_tensor("wo", [HS, CH], bf16, kind="ExternalInput").ap(),
        "bo": nc.dram_tensor("bo", [CH, 1], f32, kind="ExternalInput").ap(),
        "invc": nc.dram_tensor("invc", [CH, 1], bf16,
                               kind="ExternalInput").ap(),
        "eye": nc.dram_tensor("eye", [PT, PT], bf16, kind="ExternalInput").ap(),
        "eye32": nc.dram_tensor("eye32", [PT, PT], f32,
                                kind="ExternalInput").ap(),
        "out": nc.dram_tensor("out", [CH, T], bf16,
                              kind="ExternalOutput").ap(),
    }
    with tile.TileContext(nc) as tc:
        _kernel_body(nc, tc, d)
    nc.compile()
    return nc


_NC_CACHE = None


def _get_nc():
    global _NC_CACHE
    if _NC_CACHE is None:
        _NC_CACHE = _build_nc()
    return _NC_CACHE


def _make_in_maps(q, c, rms_w, emb, w1, b1, w2, b2, w_kv, w_out, b_out):
    q = np.asarray(q).astype(np.int64)
    c = np.asarray(c, dtype=np.float32)
    rms_w = np.asarray(rms_w, dtype=np.float32)
    emb = np.asarray(emb, dtype=np.float32)
    w1 = np.asarray(w1, dtype=np.float32)
    b1 = np.asarray(b1, dtype=np.float32)
    w2 = np.asarray(w2, dtype=np.float32)
    b2 = np.asarray(b2, dtype=np.float32)
    w_kv = np.asarray(w_kv, dtype=np.float32)
    w_out = np.asarray(w_out, dtype=np.float32)
    b_out = np.asarray(b_out, dtype=np.float32)

    # query path (tiny: 8 vectors), exact fp32 math as the reference
    qe = emb[q]                                   # [B, CH]
    x1 = qe @ w1 + b1
    h1 = x1 * (1.0 / (1.0 + np.exp(-x1)))         # silu
    qh = (h1 @ w2 + b2).reshape(B, HEADS, HD)

    wkv3 = w_kv.reshape(CH, HEADS, 2 * HD)
    w_k = wkv3[:, :, :HD]                         # [CH, HEADS, HD]
    w_v = wkv3[:, :, HD:]
    wv = np.ascontiguousarray(
        (rms_w[:, None, None] * w_v).reshape(CH, HS), dtype=np.float32)
    scale = float(HD) ** -0.5
    # wq[b, ch, e] = rms_w[ch] * scale * sum_d w_k[ch, e, d] * qh[b, e, d]
    wq_all = np.einsum("ced,bed->bce", w_k, qh).astype(np.float32)
    wq_all = wq_all * (scale * rms_w[None, :, None])

    import ml_dtypes
    bf = ml_dtypes.bfloat16
    shared = {
        "wv": wv.astype(bf),
        "wo": np.ascontiguousarray(w_out).astype(bf),
        "bo": np.ascontiguousarray(b_out.reshape(CH, 1), dtype=np.float32),
        "invc": np.full((CH, 1), 1.0 / CH, dtype=np.float32).astype(bf),
        "eye": np.eye(PT, dtype=np.float32).astype(bf),
        "eye32": np.eye(PT, dtype=np.float32),
    }
    in_maps = []
    for b in range(B):
        m = dict(shared)
        m["c"] = np.ascontiguousarray(c[b].reshape(N, CH, T)).astype(bf)
        m["wq"] = np.ascontiguousarray(wq_all[b]).astype(bf)
        in_maps.append(m)
    return in_maps


def _run(in_maps, **kwargs):
    from concourse import bass_utils

    nc = _get_nc()
    return bass_utils.run_bass_kernel_spmd(
        nc, in_maps, core_ids=list(range(NCORES)), **kwargs)


def kernel(q, c, rms_w, emb, w1, b1, w2, b2, w_kv, w_out, b_out):
    in_maps = _make_in_maps(q, c, rms_w, emb, w1, b1, w2, b2, w_kv, w_out,
                            b_out)
    res = _run(in_maps)
    outs = [np.asarray(res.results[b]["out"]).astype(np.float32)
            .reshape(CH, H, W) for b in range(B)]
    return np.stack(outs, axis=0)

